# revision 13
# baseline (speedup 1.0000x reference)
"""Hadamard transform kernel for Trainium2 (8 NeuronCores, SPMD data-parallel).

Computes y = (x @ H^T) / sqrt(D), padded with a zero imaginary plane ->
[B, S, D, 2], for x [4, 4096, 1024] fp32 and H the 1024-point Hadamard
matrix (H[i,j] = (-1)^popcount(i&j), symmetric, Kronecker-structured).

Strategy per core (shard of 2048 rows):
  H_1024 = H_8 (x) H_128  under d = a*128 + b.
  Stage 1 (PE): per 128-col chunk a, transpose x chunk (PE transpose) and
    matmul with lhsT = xT_a (the "un-transpose trick": out = lhsT.T @ rhs
    lands back in natural [n, b'] layout) against rhs = H128^T / 32.
    Products are exact: rhs entries are +-2^-5.
  Stage 2 (DVE): H_8 across the 8 chunks = 3 butterfly stages of +-adds.
    The final stage writes stride-2 into a persistent pre-zeroed SBUF out
    tile, so the zero imaginary plane costs nothing extra.
  DMA: contiguous 512 KiB loads, 1 MiB stores.
"""

import numpy as np
from contextlib import ExitStack

import concourse.bass as bass
import concourse.tile as tile
from concourse import bacc, bass_utils, mybir

N_CORES = 8
B, S, D = 4, 4096, 1024
ROWS = B * S                 # 16384
SHARD = ROWS // N_CORES      # 2048
NT = SHARD // 128            # 16 tiles of 128 rows per core
F32 = mybir.dt.float32

_cache = {}


CFG = {
    "xin_bufs": 6,
    "xt_bufs": 3,
    "w_bufs": 3,
    "n_obufs": 3,
    "pst_bufs": 2,
    "zp_bufs": 3,
    # which butterfly ops go to gpsimd (h4 ops read PSUM -> DVE only);
    # empirically (TimelineSim) any gpsimd op on the out-gating path hurts.
    "gpsimd_ops": (),
}


def _build_nc(cfg=None):
    cfg = {**CFG, **(cfg or {})}
    nc = bacc.Bacc("TRN2", target_bir_lowering=False, debug=False)
    x_d = nc.dram_tensor("x", [SHARD, D], F32, kind="ExternalInput").ap()
    r_d = nc.dram_tensor("r", [128, 128], F32, kind="ExternalInput").ap()
    i_d = nc.dram_tensor("ident", [128, 128], F32, kind="ExternalInput").ap()
    o_d = nc.dram_tensor("out", [SHARD, 2 * D], F32, kind="ExternalOutput").ap()

    def eng(name):
        return nc.gpsimd if name in cfg["gpsimd_ops"] else nc.vector

    with tile.TileContext(nc) as tc, ExitStack() as ctx:
        const_pool = ctx.enter_context(tc.tile_pool(name="const", bufs=1))
        xin_pool = ctx.enter_context(tc.tile_pool(name="xin", bufs=cfg["xin_bufs"]))
        xt_pool = ctx.enter_context(tc.tile_pool(name="xt", bufs=cfg["xt_bufs"]))
        w_pool = ctx.enter_context(tc.tile_pool(name="w", bufs=cfg["w_bufs"]))
        out_pool = ctx.enter_context(tc.tile_pool(name="outp", bufs=1))
        ps_t = ctx.enter_context(
            tc.tile_pool(name="ps_t", bufs=cfg["pst_bufs"], space="PSUM"))
        ps_z = ctx.enter_context(
            tc.tile_pool(name="ps_z", bufs=cfg["zp_bufs"], space="PSUM"))

        R_sb = const_pool.tile([128, 128], F32, tag="R")
        nc.sync.dma_start(R_sb[:], r_d[:])
        I_sb = const_pool.tile([128, 128], F32, tag="I")
        nc.sync.dma_start(I_sb[:], i_d[:])

        # Persistent output buffers; odd (imag) columns stay zero forever.
        obufs = []
        for k in range(cfg["n_obufs"]):
            ob = out_pool.tile([128, 2 * D], F32, tag=f"ob{k}")
            nc.gpsimd.memset(ob[:], 0.0)
            obufs.append(ob)

        for it in range(NT):
            x_sb = xin_pool.tile([128, D], F32, tag="x")
            nc.sync.dma_start(x_sb[:], x_d[it * 128:(it + 1) * 128, :])

            xt_sb = xt_pool.tile([128, D], F32, tag="xt")
            zp = ps_z.tile([128, D], F32, tag="zp")
            for h in range(2):
                pst = ps_t.tile([128, 512], F32, tag="pst")
                for j in range(4):
                    a = 4 * h + j
                    nc.tensor.transpose(
                        pst[:, j * 128:(j + 1) * 128],
                        x_sb[:, a * 128:(a + 1) * 128],
                        I_sb[:],
                    )
                nc.scalar.copy(xt_sb[:, h * 512:(h + 1) * 512], pst[:])
                for j in range(4):
                    a = 4 * h + j
                    nc.tensor.matmul(
                        zp[:, a * 128:(a + 1) * 128],
                        lhsT=xt_sb[:, a * 128:(a + 1) * 128],
                        rhs=R_sb[:],
                        start=True,
                        stop=True,
                    )

            # h4: chunk-distance 4. HW allows only one PSUM input per DVE op,
            # so stage the high half through SBUF via ACT first.
            zhi = xt_pool.tile([128, 512], F32, tag="zhi")
            nc.scalar.copy(zhi[:], zp[:, 512:1024])
            w1 = w_pool.tile([128, D], F32, tag="w1")
            nc.vector.tensor_add(w1[:, 0:512], zp[:, 0:512], zhi[:])
            nc.vector.tensor_sub(w1[:, 512:1024], zp[:, 0:512], zhi[:])

            # h2: chunk-distance 2
            w2 = w_pool.tile([128, D], F32, tag="w2")
            w1v = w1[:].rearrange("p (q pair c) -> p q pair c", q=2, pair=2)
            w2v = w2[:].rearrange("p (q pair c) -> p q pair c", q=2, pair=2)
            eng("h2p").tensor_add(w2v[:, :, 0, :], w1v[:, :, 0, :], w1v[:, :, 1, :])
            eng("h2m").tensor_sub(w2v[:, :, 1, :], w1v[:, :, 0, :], w1v[:, :, 1, :])

            # h1: adjacent pairs, split per half so each output half can DMA
            # out as soon as it is ready
            ob = obufs[it % cfg["n_obufs"]]
            for h in range(2):
                w2h = w2[:, h * 512:(h + 1) * 512].rearrange(
                    "p (g pair c) -> p g pair c", g=2, pair=2)
                obh = ob[:, h * 1024:(h + 1) * 1024].rearrange(
                    "p (g c two) -> p g c two", g=2, two=2)
                eng(f"h1p{h}").tensor_add(
                    obh[:, :, 0:128, 0], w2h[:, :, 0, :], w2h[:, :, 1, :]
                )
                eng(f"h1m{h}").tensor_sub(
                    obh[:, :, 128:256, 0], w2h[:, :, 0, :], w2h[:, :, 1, :]
                )
                nc.sync.dma_start(
                    o_d[it * 128:(it + 1) * 128, h * 1024:(h + 1) * 1024],
                    ob[:, h * 1024:(h + 1) * 1024],
                )

    nc.compile()
    return nc


def _get_nc():
    if "nc" not in _cache:
        _cache["nc"] = _build_nc()
    return _cache["nc"]


def kernel(x, H, **_ignored):
    x = np.asarray(x, dtype=np.float32)
    H = np.asarray(H, dtype=np.float32)
    nc = _get_nc()

    # Derive the H128 factor from the given H (exact when H has the
    # Kronecker Hadamard structure), fold in the 1/sqrt(1024) scale.
    R = np.ascontiguousarray(H[:128, :128].T) * np.float32(1.0 / 32.0)
    ident = np.eye(128, dtype=np.float32)

    xf = np.ascontiguousarray(x.reshape(ROWS, D))
    in_maps = []
    for c in range(N_CORES):
        in_maps.append({
            "x": np.ascontiguousarray(xf[c * SHARD:(c + 1) * SHARD]),
            "r": R,
            "ident": ident,
        })

    res = bass_utils.run_bass_kernel_spmd(nc, in_maps, core_ids=list(range(N_CORES)))
    outs = [res.results[c]["out"].reshape(SHARD, D, 2) for c in range(N_CORES)]
    y = np.concatenate(outs, axis=0).reshape(B, S, D, 2)
    return y.astype(np.float32)


# revision 15
# speedup vs baseline: 1.0565x; 1.0565x over previous
"""Hadamard transform kernel for Trainium2 (8 NeuronCores, SPMD data-parallel).

Computes y = (x @ H^T) / sqrt(D), padded with a zero imaginary plane ->
[B, S, D, 2], for x [4, 4096, 1024] fp32 and H the 1024-point Hadamard
matrix (H[i,j] = (-1)^popcount(i&j), symmetric, Kronecker-structured).

Strategy per core (shard of 2048 rows):
  H_1024 = H_8 (x) H_128  under d = a*128 + b.
  Stage 1 (PE): per 128-col chunk a, transpose x chunk (PE transpose) and
    matmul with lhsT = xT_a (the "un-transpose trick": out = lhsT.T @ rhs
    lands back in natural [n, b'] layout) against rhs = H128^T / 32.
    Products are exact: rhs entries are +-2^-5.
  Stage 2 (DVE): H_8 across the 8 chunks = 3 butterfly stages of +-adds.
    The final stage writes stride-2 into a persistent pre-zeroed SBUF out
    tile, so the zero imaginary plane costs nothing extra.
  DMA: contiguous 512 KiB loads, 1 MiB stores.
"""

import numpy as np
from contextlib import ExitStack

import concourse.bass as bass
import concourse.tile as tile
from concourse import bacc, bass_utils, mybir

N_CORES = 8
B, S, D = 4, 4096, 1024
ROWS = B * S                 # 16384
SHARD = ROWS // N_CORES      # 2048
NT = SHARD // 128            # 16 tiles of 128 rows per core
F32 = mybir.dt.float32

_cache = {}


CFG = {
    "xin_bufs": 6,
    "xt_bufs": 3,
    "w_bufs": 3,
    "n_obufs": 3,
    "pst_bufs": 2,
    "zp_bufs": 3,
    # which butterfly ops go to gpsimd (h4 ops read PSUM -> DVE only);
    # empirically (TimelineSim) any gpsimd op on the out-gating path hurts.
    "gpsimd_ops": (),
    "h2_split": True,
}


def _build_nc(cfg=None):
    cfg = {**CFG, **(cfg or {})}
    nc = bacc.Bacc("TRN2", target_bir_lowering=False, debug=False)
    x_d = nc.dram_tensor("x", [SHARD, D], F32, kind="ExternalInput").ap()
    r_d = nc.dram_tensor("r", [128, 128], F32, kind="ExternalInput").ap()
    i_d = nc.dram_tensor("ident", [128, 128], F32, kind="ExternalInput").ap()
    o_d = nc.dram_tensor("out", [SHARD, 2 * D], F32, kind="ExternalOutput").ap()

    def eng(name):
        return nc.gpsimd if name in cfg["gpsimd_ops"] else nc.vector

    with tile.TileContext(nc) as tc, ExitStack() as ctx:
        const_pool = ctx.enter_context(tc.tile_pool(name="const", bufs=1))
        xin_pool = ctx.enter_context(tc.tile_pool(name="xin", bufs=cfg["xin_bufs"]))
        xt_pool = ctx.enter_context(tc.tile_pool(name="xt", bufs=cfg["xt_bufs"]))
        w_pool = ctx.enter_context(tc.tile_pool(name="w", bufs=cfg["w_bufs"]))
        out_pool = ctx.enter_context(tc.tile_pool(name="outp", bufs=1))
        ps_t = ctx.enter_context(
            tc.tile_pool(name="ps_t", bufs=cfg["pst_bufs"], space="PSUM"))
        ps_z = ctx.enter_context(
            tc.tile_pool(name="ps_z", bufs=cfg["zp_bufs"], space="PSUM"))

        R_sb = const_pool.tile([128, 128], F32, tag="R")
        nc.sync.dma_start(R_sb[:], r_d[:])
        I_sb = const_pool.tile([128, 128], F32, tag="I")
        nc.sync.dma_start(I_sb[:], i_d[:])

        # Persistent output buffers; odd (imag) columns stay zero forever.
        obufs = []
        for k in range(cfg["n_obufs"]):
            ob = out_pool.tile([128, 2 * D], F32, tag=f"ob{k}")
            nc.gpsimd.memset(ob[:], 0.0)
            obufs.append(ob)

        for it in range(NT):
            x_sb = xin_pool.tile([128, D], F32, tag="x")
            nc.sync.dma_start(x_sb[:], x_d[it * 128:(it + 1) * 128, :])

            xt_sb = xt_pool.tile([128, D], F32, tag="xt")
            zp = ps_z.tile([128, D], F32, tag="zp")
            for h in range(2):
                pst = ps_t.tile([128, 512], F32, tag="pst")
                for j in range(4):
                    a = 4 * h + j
                    nc.tensor.transpose(
                        pst[:, j * 128:(j + 1) * 128],
                        x_sb[:, a * 128:(a + 1) * 128],
                        I_sb[:],
                    )
                nc.scalar.copy(xt_sb[:, h * 512:(h + 1) * 512], pst[:])
                for j in range(4):
                    a = 4 * h + j
                    nc.tensor.matmul(
                        zp[:, a * 128:(a + 1) * 128],
                        lhsT=xt_sb[:, a * 128:(a + 1) * 128],
                        rhs=R_sb[:],
                        start=True,
                        stop=True,
                    )

            # h4: chunk-distance 4. HW allows only one PSUM input per DVE op,
            # so stage the high half through SBUF via ACT first.
            zhi = xt_pool.tile([128, 512], F32, tag="zhi")
            nc.scalar.copy(zhi[:], zp[:, 512:1024])
            w1 = w_pool.tile([128, D], F32, tag="w1")
            nc.vector.tensor_add(w1[:, 0:512], zp[:, 0:512], zhi[:])
            nc.vector.tensor_sub(w1[:, 512:1024], zp[:, 0:512], zhi[:])

            # h2: chunk-distance 2 (half-local; split per half when configured)
            w2 = w_pool.tile([128, D], F32, tag="w2")
            if cfg.get("h2_split"):
                for h in range(2):
                    w1h = w1[:, h * 512:(h + 1) * 512].rearrange(
                        "p (pair c) -> p pair c", pair=2)
                    w2h = w2[:, h * 512:(h + 1) * 512].rearrange(
                        "p (pair c) -> p pair c", pair=2)
                    eng("h2p").tensor_add(w2h[:, 0, :], w1h[:, 0, :], w1h[:, 1, :])
                    eng("h2m").tensor_sub(w2h[:, 1, :], w1h[:, 0, :], w1h[:, 1, :])
            else:
                w1v = w1[:].rearrange("p (q pair c) -> p q pair c", q=2, pair=2)
                w2v = w2[:].rearrange("p (q pair c) -> p q pair c", q=2, pair=2)
                eng("h2p").tensor_add(
                    w2v[:, :, 0, :], w1v[:, :, 0, :], w1v[:, :, 1, :])
                eng("h2m").tensor_sub(
                    w2v[:, :, 1, :], w1v[:, :, 0, :], w1v[:, :, 1, :])

            # h1: adjacent pairs, split per half so each output half can DMA
            # out as soon as it is ready
            ob = obufs[it % cfg["n_obufs"]]
            for h in range(2):
                w2h = w2[:, h * 512:(h + 1) * 512].rearrange(
                    "p (g pair c) -> p g pair c", g=2, pair=2)
                obh = ob[:, h * 1024:(h + 1) * 1024].rearrange(
                    "p (g c two) -> p g c two", g=2, two=2)
                eng(f"h1p{h}").tensor_add(
                    obh[:, :, 0:128, 0], w2h[:, :, 0, :], w2h[:, :, 1, :]
                )
                eng(f"h1m{h}").tensor_sub(
                    obh[:, :, 128:256, 0], w2h[:, :, 0, :], w2h[:, :, 1, :]
                )
                nc.sync.dma_start(
                    o_d[it * 128:(it + 1) * 128, h * 1024:(h + 1) * 1024],
                    ob[:, h * 1024:(h + 1) * 1024],
                )

    nc.compile()
    return nc


def _get_nc():
    if "nc" not in _cache:
        _cache["nc"] = _build_nc()
    return _cache["nc"]


def kernel(x, H, **_ignored):
    x = np.asarray(x, dtype=np.float32)
    H = np.asarray(H, dtype=np.float32)
    nc = _get_nc()

    # Derive the H128 factor from the given H (exact when H has the
    # Kronecker Hadamard structure), fold in the 1/sqrt(1024) scale.
    R = np.ascontiguousarray(H[:128, :128].T) * np.float32(1.0 / 32.0)
    ident = np.eye(128, dtype=np.float32)

    xf = np.ascontiguousarray(x.reshape(ROWS, D))
    in_maps = []
    for c in range(N_CORES):
        in_maps.append({
            "x": np.ascontiguousarray(xf[c * SHARD:(c + 1) * SHARD]),
            "r": R,
            "ident": ident,
        })

    res = bass_utils.run_bass_kernel_spmd(nc, in_maps, core_ids=list(range(N_CORES)))
    outs = [res.results[c]["out"].reshape(SHARD, D, 2) for c in range(N_CORES)]
    y = np.concatenate(outs, axis=0).reshape(B, S, D, 2)
    return y.astype(np.float32)
